# revision 37
# baseline (speedup 1.0000x reference)
"""Multi-head causal attention (no output proj) on 8 TRN2 NeuronCores.

Problem: x[2,2048,2048] fp32, Wq/Wk/Wv[2048,2048] fp32, 16 heads of dim 128,
causal mask (fill -1e6), softmax, out = attn @ v -> [2,2048,2048] fp32.

Sharding: tensor-parallel over heads. Core c owns heads (2c, 2c+1) for both
batches: it computes Q/K/V projections for its 256 output columns and full
attention for its 4 (batch, head) instances, writing output columns
[256c : 256c+256]. No collectives.

Dataflow per core (all matmul operands fp16, PSUM accumulation fp32):
  - host supplies x^T and W slices pre-tiled to SBUF layout (fp16,
    contiguous per partition line -> large DMA packets), plus causal masks.
  - Projections: QT/KT [e, s] = W.T @ x.T per head (lhsT = W chunk, rhs = xT
    chunk); V [s, e] natural (lhsT = xT chunk, rhs = Wv chunk), stored with a
    ones column appended per head so the attn @ V matmul also produces the
    softmax denominator for free.
  - Scores, transposed: S^T[j, i] = matmul(lhsT=KT j-tile, rhs=QT i-block).
    Softmax without max-subtraction (scores ~ N(0,1); masked -> exp * 0).
    exp on ScalarE (scale=1/sqrt(128) fused), output fp16.
  - attn @ V: O[i, e+1] = sum_j matmul(lhsT=P^T tile, rhs=[V_h | ones]);
    col 128 = row sum. Normalize with per-partition reciprocal scale.

Schedule: block 0 is emitted chunk-major (q0/q1/k0/k1 matmuls interleaved
per x chunk, 4 concurrent PSUM accumulation groups) so the PE consumes each
128 KB x chunk over ~850 ns instead of ~210 ns and never outruns the DMA
warmup; its x/W chunks are loaded as single-chunk DMAs in consumption
order. Attention thunks drain at up to 3 per projection slot so the
ScalarE-bound exp backlog doesn't pile up into the endgame. Output is
written fp16 with both heads merged per i-tile, DMA split across two
queues by partition halves (host converts back to fp32).
"""

import math

import numpy as np

import concourse.mybir as mybir
import concourse.tile as tile
from concourse import bacc
from concourse.bass_utils import run_bass_kernel_spmd

# ---- problem constants (hardcoded; kernel.py must be self-contained) ----
D = 2048            # model dim (contraction for projections)
S = 2048            # sequence length per batch
NB = 2              # batches
H = 2               # heads per core
E = 128             # head dim
N_CORES = 8
IBLK = 512          # i-block (query block, matmul free dim)
JT = 128            # j-tile (key tile, partition dim)
P = 128             # partitions

FP16 = mybir.dt.float16
FP32 = mybir.dt.float32


def build_program(d=D, s=S, nb=NB, h=H, e=E, iblk=IBLK):
    """Build the per-core Bass program. Returns (nc, names dict)."""
    kd = d // P                 # contraction chunks
    st = nb * s                 # total rows of x (batches flattened)
    n_sblk = st // iblk         # projection s-blocks
    sb_per_batch = s // iblk    # i-blocks per batch
    jt_per_batch = s // JT      # j-tiles per batch
    it_per_blk = iblk // P      # i-tiles per i-block
    ew = h * e                  # projection output width per core (both heads)
    vw = h * (e + 1)            # V tile width incl. ones columns

    nc = bacc.Bacc(None, target_bir_lowering=False)
    names = {}

    with tile.TileContext(nc) as tc:
        with tc.tile_pool(name="dram", bufs=1, space="DRAM") as dram:
            # host-pre-tiled layouts (contiguous per SBUF partition line, so
            # DMA moves large packets): xTt[blk, p, t, c] = x[blk*iblk+c,
            # t*128+p]; w[p, t, e] = W[t*128+p, head cols]
            xTt = dram.tile([n_sblk, P, kd, iblk], FP16, kind="ExternalInput")
            # wall[p, t, :] = [wq cols | wk cols | wv cols] for chunk t:
            # per-(partition, chunk) contiguity 1.5KB, and a single stream
            # delivers all three weights in exact chunk-need order.
            wall = dram.tile([P, kd, 3 * ew], FP16, kind="ExternalInput")
            msk = dram.tile([P, 3 * iblk], FP16, kind="ExternalInput")
            out = dram.tile([st, ew], FP16, kind="ExternalOutput")
            names.update(xT=xTt.name, wall=wall.name,
                         msk=msk.name, out=out.name)

            with (
                tc.tile_pool(name="wpool", bufs=1) as wpool,
                tc.tile_pool(name="xpool", bufs=4) as xpool,
                tc.tile_pool(name="qkv", bufs=1) as qkv,
                tc.tile_pool(name="ppool", bufs=16) as ppool,
                tc.tile_pool(name="opool", bufs=6) as opool,
                tc.tile_pool(name="psA", bufs=2, space="PSUM") as psA,
                tc.tile_pool(name="psB", bufs=2, space="PSUM") as psB,
            ):
                # ---- startup loads, multi-engine issue, need-order ----
                # The 16 DMA engine contexts are shared by all queues and
                # each tops out at ~20-24 GB/s once packets are >=2KB, so
                # aggregate bandwidth is set by average in-flight packet
                # size, not queue count. Streams escalate part sizes:
                # small leading parts for low first-need latency, then
                # 2-8KB-contiguous parts for the bulk, spread over the
                # three DMA-capable sequencers (sync/scalar/gpsimd) in
                # consumption order.
                xtb0 = xpool.tile([P, kd, iblk], FP16, tag="xT", name="xtb0")
                xtb1 = xpool.tile([P, kd, iblk], FP16, tag="xT", name="xtb1")
                w_sb = wpool.tile([P, kd, 3 * ew], FP16, tag="wall",
                                  name="wall_sb")
                # column offsets of wq / wk / wv inside a wall chunk line
                wof = {"q": 0, "k": ew, "v": 2 * ew}
                mask_sb = wpool.tile([P, 3 * iblk], FP16, tag="mask")

                # sync: all of x block-0 in escalating parts whose lumped
                # completion times track the ~1.28us/chunk consumption
                # deadlines of the fused block-0 loop, then the mask and
                # the first half of xtb1. All queues start transferring at
                # ~7.2us (end of the engine preamble).
                for t0, t1 in ((0, 1), (1, 2), (2, 4), (4, 6), (6, 8),
                               (8, 10), (10, 12), (12, 16)):
                    nc.sync.dma_start(out=xtb0[:, t0:t1, :],
                                      in_=xTt[0, :, t0:t1, :])
                nc.sync.dma_start(out=mask_sb, in_=msk[:])
                nc.sync.dma_start(out=xtb1[:, 0:kd // 2, :],
                                  in_=xTt[1, :, 0:kd // 2, :])
                # wall chunks alternate scalar/gpsimd in chunk order: each
                # queue owes only 192KB per 2.56us -- well under a queue's
                # ~95-110 B/ns share -- so wq/wk/wv never gate a chunk.
                # Chunk 0 is split (wq|wk on scalar, wv on gpsimd) so the
                # first qk matmul waits on only 128KB.
                nc.scalar.dma_start(out=w_sb[:, 0:1, 0:2 * ew],
                                    in_=wall[:, 0:1, 0:2 * ew])
                nc.gpsimd.dma_start(out=w_sb[:, 0:1, 2 * ew:3 * ew],
                                    in_=wall[:, 0:1, 2 * ew:3 * ew])
                for t in range(1, kd):
                    eng = nc.scalar if t % 2 == 0 else nc.gpsimd
                    eng.dma_start(out=w_sb[:, t:t + 1, :],
                                  in_=wall[:, t:t + 1, :])
                nc.gpsimd.dma_start(out=xtb1[:, kd // 2:kd, :],
                                    in_=xTt[1, :, kd // 2:kd, :])

                def load_xtb(blk, parts=2):
                    t = xpool.tile([P, kd, iblk], FP16, tag="xT",
                                   name=f"xtb{blk}")
                    step = kd // parts
                    for q in range(parts):
                        t0, t1 = q * step, (q + 1) * step
                        nc.sync.dma_start(out=t[:, t0:t1, :],
                                          in_=xTt[blk, :, t0:t1, :])
                    return t

                # ---- persistent QT/KT/V in SBUF (fp16) ----
                # qT/kT: per head, [e, st] with batches side by side.
                qT = [qkv.tile([P, st], FP16, tag=f"qT{i}", name=f"qT{i}")
                      for i in range(h)]
                kT = [qkv.tile([P, st], FP16, tag=f"kT{i}", name=f"kT{i}")
                      for i in range(h)]
                # V: per (batch, j-tile): [128 j, h*(e+1)] with ones columns.
                vt = [qkv.tile([P, vw], FP16, tag=f"v{i}", name=f"v{i}")
                      for i in range(nb * jt_per_batch)]

                # ---- PE clock pre-warm ----
                # The tensor engine's DVFS ramps 0.65 -> 1.2 -> 2.4 GHz over
                # ~3us of continuous execution. Real work can't start until
                # the first x/w chunks land (~9.5us, just after the ~7.2us
                # engine preamble), so burn that wait on dependency-free
                # dummy matmuls. All accumulate into ONE psum tile in one
                # accumulation group so they stream back-to-back instead of
                # serializing on psum-buffer reuse (~213ns each).
                warm = wpool.tile([P, ew], FP16, tag="warm")
                nc.vector.memset(warm[:], 0.5)
                pw = psB.tile([P, ew], FP32, tag="psB", name="pw")
                n_warm = 12
                for i in range(n_warm):
                    nc.tensor.matmul(pw[:], warm[:, 0:P], warm[:],
                                     start=(i == 0), stop=(i == n_warm - 1))

                # ---- block 0: fully fused chunk-major qkv ----
                # 8 concurrent PSUM accumulation chains (q0,q1,k0 in psq;
                # k1 + v-it0 + v-it1 packed into psk's three banks; v-it2/
                # v-it3 in psB) consume each x chunk for 3072 PE cycles.
                # That drops the startup DMA demand to ~250 B/ns (x is
                # reused by the v matmuls), under the ~290 B/ns the queues
                # deliver: no stall, so the DVFS clock never drops to 1.2
                # GHz mid-block (a ~2us stall costs ~6us of half-clock).
                def emit_block0():
                    psq = psA.tile([P, 3 * iblk], FP32, tag="psA",
                                   name="ps0a")
                    psk = psA.tile([P, 3 * iblk], FP32, tag="psA",
                                   name="ps0b")
                    pv2 = psB.tile([P, ew], FP32, tag="psB", name="pv2")
                    pv3 = psB.tile([P, ew], FP32, tag="psB", name="pv3")
                    first, last = 0, kd - 1
                    for t in range(kd):
                        st_, sp_ = t == first, t == last
                        # q0 | q1 | k0 (one bank-chain each in psq)
                        for g, (name, c0) in enumerate(
                                (("q", 0), ("q", e), ("k", 0))):
                            nc.tensor.matmul(
                                psq[:, g * iblk:(g + 1) * iblk],
                                w_sb[:, t, wof[name] + c0:
                                     wof[name] + c0 + e],
                                xtb0[:, t, :],
                                start=st_, stop=sp_,
                            )
                        # k1 -> psk bank 0
                        nc.tensor.matmul(
                            psk[:, 0:iblk],
                            w_sb[:, t, wof["k"] + e:wof["k"] + 2 * e],
                            xtb0[:, t, :],
                            start=st_, stop=sp_,
                        )
                        # v it0 + it1 share psk bank 1 (one chain: start on
                        # the first it0 write, stop on the last it1 write,
                        # disjoint 256-col regions -- diag-quad pattern)
                        nc.tensor.matmul(
                            psk[:, iblk:iblk + ew],
                            xtb0[:, t, 0:P],
                            w_sb[:, t, wof["v"]:wof["v"] + ew],
                            start=st_, stop=False,
                        )
                        nc.tensor.matmul(
                            psk[:, iblk + ew:iblk + 2 * ew],
                            xtb0[:, t, P:2 * P],
                            w_sb[:, t, wof["v"]:wof["v"] + ew],
                            start=False, stop=sp_,
                        )
                        # v it2 / it3 in psB tiles
                        nc.tensor.matmul(
                            pv2[:], xtb0[:, t, 2 * P:3 * P],
                            w_sb[:, t, wof["v"]:wof["v"] + ew],
                            start=st_, stop=sp_,
                        )
                        nc.tensor.matmul(
                            pv3[:], xtb0[:, t, 3 * P:4 * P],
                            w_sb[:, t, wof["v"]:wof["v"] + ew],
                            start=st_, stop=sp_,
                        )
                    # evacuation copies split scalar/vector so the psum
                    # buffers free in ~2us instead of ~8us of serial
                    # vector CASTs (block-1's projections WAR on them).
                    # pv2/pv3 first on vector: block-1 starts with vprojs
                    # whose psB buffers WAR on exactly those.
                    vsrc = [psk[:, iblk:iblk + ew],
                            psk[:, iblk + ew:iblk + 2 * ew],
                            pv2[:], pv3[:]]
                    for it in (2, 3, 0, 1):
                        v_dst = vt[it]
                        for hh in range(h):
                            nc.vector.tensor_copy(
                                v_dst[:, hh * (e + 1):hh * (e + 1) + e],
                                vsrc[it][:, hh * e:(hh + 1) * e],
                            )
                            nc.vector.memset(
                                v_dst[:, hh * (e + 1) + e:
                                      hh * (e + 1) + e + 1],
                                1.0,
                            )
                    cp = mybir.ActivationFunctionType.Copy
                    nc.scalar.activation(qT[0][:, 0:iblk], psq[:, 0:iblk],
                                         cp)
                    nc.scalar.activation(qT[1][:, 0:iblk],
                                         psq[:, iblk:2 * iblk], cp)
                    nc.scalar.activation(kT[0][:, 0:iblk],
                                         psq[:, 2 * iblk:3 * iblk], cp)
                    nc.scalar.activation(kT[1][:, 0:iblk], psk[:, 0:iblk],
                                         cp)

                # ---- projections for blocks 1+, as per-chain thunks ----
                def proj_thunks(blk, pre=None):
                    s0 = blk * iblk
                    box = {}
                    if pre is not None:
                        box["x"] = pre

                    def qk(name, hh):
                        def f():
                            if "x" not in box:
                                box["x"] = load_xtb(blk)
                            xtb = box["x"]
                            dst = {"q": qT, "k": kT}[name][hh]
                            ps = psA.tile([P, iblk], FP32, tag="psA",
                                          name="ps")
                            for t in range(kd):
                                nc.tensor.matmul(
                                    ps[:],
                                    w_sb[:, t, wof[name] + hh * e:
                                         wof[name] + (hh + 1) * e],
                                    xtb[:, t, :],
                                    start=(t == 0),
                                    stop=(t == kd - 1),
                                )
                            # scalar evacuates qk psum (vector is the
                            # busier engine); the last block keeps vector
                            # so scalar's endgame exp chain stays clear
                            if blk < n_sblk - 1:
                                nc.scalar.activation(
                                    dst[:, s0:s0 + iblk], ps[:],
                                    mybir.ActivationFunctionType.Copy)
                            else:
                                nc.vector.tensor_copy(dst[:, s0:s0 + iblk],
                                                      ps[:])
                        return f

                    def vproj(it):
                        def f():
                            if "x" not in box:
                                box["x"] = load_xtb(blk)
                            xtb = box["x"]
                            ps = psB.tile([P, ew], FP32, tag="psB", name="ps")
                            for t in range(kd):
                                nc.tensor.matmul(
                                    ps[:],
                                    xtb[:, t, it * P:(it + 1) * P],
                                    w_sb[:, t, wof["v"]:wof["v"] + ew],
                                    start=(t == 0),
                                    stop=(t == kd - 1),
                                )
                            v_dst = vt[(s0 + it * P) // JT]
                            for hh in range(h):
                                nc.vector.tensor_copy(
                                    v_dst[:, hh * (e + 1):hh * (e + 1) + e],
                                    ps[:, hh * e:(hh + 1) * e],
                                )
                                nc.vector.memset(
                                    v_dst[:, hh * (e + 1) + e:
                                          hh * (e + 1) + e + 1],
                                    1.0,
                                )
                        return f

                    return ([[qk(n, hh) for n in ("q", "k")]
                             for hh in range(h)],
                            [vproj(it) for it in range(it_per_blk)])

                # ---- attention, software-pipelined ----
                # For each (batch, head, i-block) step: scores+exp for step
                # k+1 are emitted before the attn@V matmuls of step k, so the
                # PE never stalls waiting on ScalarE's exp.
                inv_sqrt_e = 1.0 / math.sqrt(e)

                def scores_thunks(b, hh, ib, p_tiles):
                    """Per-group thunks for one i-block's scores+exp+mask.

                    Full j-tiles go three-per-PSUM-tile (3 banks; one wide
                    exp covers all three). The 4 narrowed diagonal j-tiles
                    pack into ONE 3-bank tile: bank0 = d0[512], bank1 =
                    d1[384] + d3[128] (one accumulation group, disjoint
                    writes), bank2 = d2[256]; a single exp + one host-built
                    mask handle the whole diagonal. Each thunk appends
                    per-jt (p_tile, eff) entries to p_tiles; the PV lhsT
                    slice for i-tile t is p_tile[:, t*128+eff :][:128].
                    """
                    i0 = b * s + ib * iblk
                    n_full = it_per_blk * ib

                    def score_mm(sp, base, jt, c0, start=True, stop=True):
                        nc.tensor.matmul(
                            sp[:, base:base + (iblk - c0)],
                            kT[hh][:, b * s + jt * JT:b * s + (jt + 1) * JT],
                            qT[hh][:, i0 + c0:i0 + iblk],
                            start=start,
                            stop=stop,
                        )

                    def full_group(g0):
                        def f():
                            gn = min(3, n_full - g0)
                            sp = psA.tile([P, 3 * iblk], FP32, tag="psA",
                                          name="sp")
                            pt = ppool.tile([P, 3 * iblk], FP16, tag="p",
                                            name="pt")
                            for k in range(gn):
                                score_mm(sp, k * iblk, g0 + k, 0)
                                p_tiles.append((pt, k * iblk))
                            nc.scalar.activation(
                                pt[:, 0:gn * iblk], sp[:, 0:gn * iblk],
                                mybir.ActivationFunctionType.Exp,
                                scale=inv_sqrt_e,
                            )
                        return f

                    def diag_quad():
                        sp = psA.tile([P, 3 * iblk], FP32, tag="psA",
                                      name="sp")
                        pt = ppool.tile([P, 3 * iblk], FP16, tag="p",
                                        name="pt")
                        q0 = n_full
                        score_mm(sp, 0, q0 + 0, 0)                # d0 [0:512]
                        score_mm(sp, iblk, q0 + 1, P, stop=False)
                        score_mm(sp, iblk + 384, q0 + 3, 3 * P,
                                 start=False)                     # d3
                        score_mm(sp, 2 * iblk, q0 + 2, 2 * P)     # d2
                        p_tiles.append((pt, 0))                   # d0: eff 0
                        p_tiles.append((pt, iblk - P))            # d1: eff 384
                        p_tiles.append((pt, 2 * iblk - 2 * P))    # d2: eff 768
                        p_tiles.append((pt, iblk + 384 - 3 * P))  # d3: eff 512
                        tw = 2 * iblk + 256
                        nc.scalar.activation(
                            pt[:, 0:tw], sp[:, 0:tw],
                            mybir.ActivationFunctionType.Exp,
                            scale=inv_sqrt_e,
                        )
                        nc.vector.tensor_mul(
                            pt[:, 0:tw], pt[:, 0:tw], mask_sb[:, 0:tw]
                        )

                    return ([full_group(g0) for g0 in range(0, n_full, 3)]
                            + [diag_quad])

                # merged per-i-tile output tiles: both heads' normalized
                # rows land in one [128, 256] fp16 tile, DMA'd once (split
                # across two queues by partition halves).
                obt_map = {}

                def pv_thunks(b, hh, ib, p_tiles):
                    i0 = b * s + ib * iblk
                    jbase = b * jt_per_batch

                    def one(it):
                        def f():
                            op = psB.tile([P, e + 1], FP32, tag="psB",
                                          name="op")
                            last = it_per_blk * ib + it
                            for jt in range(last + 1):
                                pt, eff = p_tiles[jt]
                                lo = it * P + eff
                                nc.tensor.matmul(
                                    op[:],
                                    pt[:, lo:lo + P],
                                    vt[jbase + jt][:, hh * (e + 1):
                                                   (hh + 1) * (e + 1)],
                                    start=(jt == 0),
                                    stop=(jt == last),
                                )
                            rec = opool.tile([P, 1], FP32, tag="rec",
                                             name="rec")
                            key = (b, ib, it)
                            ent = obt_map.get(key)
                            if ent is None:
                                obt = opool.tile([P, ew], FP16, tag="obt",
                                                 bufs=12, name="obt")
                                ent = obt_map[key] = [obt, 0]
                            obt = ent[0]
                            nc.vector.reciprocal(rec[:], op[:, e:e + 1])
                            nc.vector.tensor_scalar_mul(
                                obt[:, hh * e:(hh + 1) * e], op[:, 0:e],
                                rec[:])
                            ent[1] += 1
                            if ent[1] == h:
                                r0 = i0 + it * P
                                # the last i-block's tiles go on sync +
                                # scalar in halves (queues empty by then,
                                # HW drains ~10ns, parallel transfer);
                                # everything earlier stays on gpsimd,
                                # whose software-DGE drain polls ~3us past
                                # the final transfer -- fine mid-run,
                                # deadly in the teardown.
                                if (b, ib) == (nb - 1, sb_per_batch - 1):
                                    eng = nc.sync if it % 2 == 0 else nc.scalar
                                    eng.dma_start(
                                        out=out[r0:r0 + P, :],
                                        in_=obt[:],
                                    )
                                else:
                                    nc.gpsimd.dma_start(
                                        out=out[r0:r0 + P, :],
                                        in_=obt[:],
                                    )
                                del obt_map[key]
                        return f

                    return [one(it) for it in range(it_per_blk)]

                # ---- interleaved emission ----
                # Attention step (b, hh, ib) becomes ready once projection
                # s-block b*sb_per_batch+ib is emitted. Its score-group
                # thunks are queued immediately, its attn@V thunks one step
                # later (so scores of the next step always precede attn@V of
                # the previous -> no exp-latency stall). Between projection
                # thunks, 1-3 attention thunks are emitted depending on
                # backlog, so the ScalarE-bound endgame stays small.
                from collections import deque

                attn_q = deque()
                pending_pv = None   # (thunks, step_block) of the last step
                step_list = sorted(
                    [(b, hh, ib) for b in range(nb) for hh in range(h)
                     for ib in range(sb_per_batch)],
                    key=lambda st: (st[0] * sb_per_batch + st[2], st[1]),
                )
                si = 0

                def queue_ready(blk_done, hh_done=None):
                    # admit steps whose q/k projections are emitted; with
                    # hh_done set, only heads <= hh_done of block blk_done
                    nonlocal si, pending_pv
                    def admitted(st):
                        sb = st[0] * sb_per_batch + st[2]
                        if sb < blk_done:
                            return True
                        return sb == blk_done and (hh_done is None
                                                   or st[1] <= hh_done)
                    while (si < len(step_list)
                           and admitted(step_list[si])):
                        st = step_list[si]
                        si += 1
                        st_blk = st[0] * sb_per_batch + st[2]
                        shared = []
                        sc = [("sc", t, None)
                              for t in scores_thunks(*st, shared)]
                        pv = ([("pv", t, pending_pv[1])
                               for t in pending_pv[0]]
                              if pending_pv is not None else [])
                        # zip score-groups with the previous step's attn@V
                        # thunks: each attn@V group is ~1.5us of PE work that
                        # covers the exp latency of the preceding scores.
                        # Scores advance at ~2x rate so the LAST (diag)
                        # group lands before the final attn@V thunks: its
                        # exp completes under them instead of stalling the
                        # next step's first attn@V.
                        merged = []
                        if sc and pv:
                            merged.extend([sc[0], pv[0]])
                            rest_sc, i2 = sc[1:], 0
                            for p_ in pv[1:]:
                                merged.extend(rest_sc[i2:i2 + 2])
                                i2 += 2
                                merged.append(p_)
                            merged.extend(rest_sc[i2:])
                        else:
                            merged = sc + pv
                        attn_q.extend(merged)
                        pending_pv = (pv_thunks(*st, shared), st_blk)

                def pop_some(vphase_blk=None):
                    npop = (5 if len(attn_q) > 24
                            else 4 if len(attn_q) > 16
                            else 3 if len(attn_q) > 8
                            else 2 if len(attn_q) > 2 else 1)
                    sc_run = 0
                    for _ in range(npop):
                        if not attn_q:
                            break
                        kind, fn, pv_blk = attn_q[0]
                        # cap back-to-back score groups per slot: the
                        # 2-deep psA pool stalls the PE on a 3rd
                        if kind == "sc":
                            sc_run += 1
                            if sc_run > 2:
                                break
                        # a pv whose step belongs to the block whose
                        # V-projections are being emitted right now would
                        # be ordered before the V tiles it reads
                        elif vphase_blk is not None and pv_blk == vphase_blk:
                            break
                        attn_q.popleft()
                        fn()

                # steps become ready right after their block's qk thunks:
                # their scores+exp overlap the block's V-projections, so
                # the ScalarE exp backlog starts draining ~7us earlier per
                # block and the endgame exp chain is mostly prepaid.
                emit_block0()
                queue_ready(0)
                pre_x = {1: xtb1}
                for blk in range(1, n_sblk):
                    qk_thunks, v_thunks = proj_thunks(blk,
                                                      pre=pre_x.get(blk))
                    # v it0/it1 first: their psB buffers clear fastest
                    # after the previous block, so the PE restarts without
                    # waiting on the bigger psA evacuations. Each head's
                    # attention step is admitted right after its own q/k
                    # thunks, so its scores+exp overlap the rest of the
                    # block (prepaying the ScalarE exp chain).
                    for th in v_thunks[:2]:
                        th()
                        pop_some(vphase_blk=blk)
                    for hh, pair in enumerate(qk_thunks):
                        for th in pair:
                            th()
                            pop_some()
                        queue_ready(blk, hh_done=hh)
                    for th in v_thunks[2:]:
                        th()
                        pop_some(vphase_blk=blk)
                # drain: remaining scores first (their exps are the only
                # latency left), then the last two steps' attn@V thunks
                # interleaved per i-tile so merged output tiles complete
                # and DMA progressively instead of all at the very end.
                rest = list(attn_q)
                attn_q.clear()
                for ent in rest:
                    if ent[0] == "sc":
                        ent[1]()
                last_pv = [ent[1] for ent in rest if ent[0] == "pv"]
                tail_pv = pending_pv[0] if pending_pv is not None else []
                n_ = max(len(last_pv), len(tail_pv))
                for i in range(n_):
                    if i < len(last_pv):
                        last_pv[i]()
                    if i < len(tail_pv):
                        tail_pv[i]()

    nc.compile()
    return nc, names


def host_tile_x(x_flat, iblk, p=P):
    """[st, d] -> [n_sblk, p, kd, iblk] with layout x[blk*iblk+c, t*p+pp]."""
    st, d = x_flat.shape
    return np.ascontiguousarray(
        x_flat.reshape(st // iblk, iblk, d // p, p).transpose(0, 3, 2, 1)
        .astype(np.float16)
    )


def host_tile_w(w_cols, p=P):
    """[d, ew] -> [p, kd, ew] with layout W[t*p+pp, e]."""
    d, ew = w_cols.shape
    return np.ascontiguousarray(
        w_cols.reshape(d // p, p, ew).transpose(1, 0, 2).astype(np.float16)
    )


def host_mask(iblk, p=P):
    """Causal mask [p, 3*iblk] for the packed diagonal quad layout:
    cols [0:512]=d0, [512:896]=d1(384), [896:1024]=d3(128), [1024:1280]=d2
    (256). Every narrowed diagonal tile reduces to the base pattern
    diag[pp, c] = (pp <= c)."""
    diag = (np.arange(p)[:, None] <= np.arange(iblk)[None, :])
    m = np.zeros((p, 3 * iblk), dtype=np.float16)
    m[:, 0:iblk] = diag
    m[:, iblk:iblk + 384] = diag[:, 0:384]
    m[:, iblk + 384:iblk + 512] = diag[:, 0:128]
    m[:, 2 * iblk:2 * iblk + 256] = diag[:, 0:256]
    return m


def _host_prep(x, Wq, Wk, Wv):
    """Shard + cast inputs on host. Returns list of 8 in_maps."""
    st = x.shape[0] * x.shape[1]
    xTt = host_tile_x(x.reshape(st, D), IBLK)
    msk = host_mask(IBLK)
    in_maps = []
    for c in range(N_CORES):
        cols = slice(2 * c * E, 2 * (c + 1) * E)
        wall = np.ascontiguousarray(np.concatenate(
            [host_tile_w(Wq[:, cols]), host_tile_w(Wk[:, cols]),
             host_tile_w(Wv[:, cols])], axis=2))
        in_maps.append({
            "xT": xTt,
            "wall": wall,
            "msk": msk,
        })
    return in_maps


_CACHE = {}


def _get_program():
    if "nc" not in _CACHE:
        nc, names = build_program()
        _CACHE["nc"] = nc
        _CACHE["names"] = names
    return _CACHE["nc"], _CACHE["names"]


def kernel(x, Wq, Wk, Wv, _trace=False, _tmpdir=None):
    nc, names = _get_program()
    raw_maps = _host_prep(np.asarray(x), np.asarray(Wq), np.asarray(Wk),
                          np.asarray(Wv))
    in_maps = [{names[k]: v for k, v in m.items()} for m in raw_maps]
    res = run_bass_kernel_spmd(
        nc, in_maps, core_ids=list(range(N_CORES)),
        trace=_trace, tmpdir=_tmpdir,
    )
    b, s, d = x.shape
    out = np.empty((b, s, d), dtype=np.float32)
    for c in range(N_CORES):
        core_out = res.results[c][names["out"]]  # [4096, 256] fp16
        out[:, :, 2 * c * E:2 * (c + 1) * E] = (
            core_out.astype(np.float32).reshape(b, s, 2 * E))
    if _trace:
        _CACHE["last_results"] = res
    return out



# revision 38
# speedup vs baseline: 1.0223x; 1.0223x over previous
"""Multi-head causal attention (no output proj) on 8 TRN2 NeuronCores.

Problem: x[2,2048,2048] fp32, Wq/Wk/Wv[2048,2048] fp32, 16 heads of dim 128,
causal mask (fill -1e6), softmax, out = attn @ v -> [2,2048,2048] fp32.

Sharding: tensor-parallel over heads. Core c owns heads (2c, 2c+1) for both
batches: it computes Q/K/V projections for its 256 output columns and full
attention for its 4 (batch, head) instances, writing output columns
[256c : 256c+256]. No collectives.

Dataflow per core (all matmul operands fp16, PSUM accumulation fp32):
  - host supplies x^T and W slices pre-tiled to SBUF layout (fp16,
    contiguous per partition line -> large DMA packets), plus causal masks.
  - Projections: QT/KT [e, s] = W.T @ x.T per head (lhsT = W chunk, rhs = xT
    chunk); V [s, e] natural (lhsT = xT chunk, rhs = Wv chunk), stored with a
    ones column appended per head so the attn @ V matmul also produces the
    softmax denominator for free.
  - Scores, transposed: S^T[j, i] = matmul(lhsT=KT j-tile, rhs=QT i-block).
    Softmax without max-subtraction (scores ~ N(0,1); masked -> exp * 0).
    exp on ScalarE (scale=1/sqrt(128) fused), output fp16.
  - attn @ V: O[i, e+1] = sum_j matmul(lhsT=P^T tile, rhs=[V_h | ones]);
    col 128 = row sum. Normalize with per-partition reciprocal scale.

Schedule: block 0 is emitted chunk-major (q0/q1/k0/k1 matmuls interleaved
per x chunk, 4 concurrent PSUM accumulation groups) so the PE consumes each
128 KB x chunk over ~850 ns instead of ~210 ns and never outruns the DMA
warmup; its x/W chunks are loaded as single-chunk DMAs in consumption
order. Attention thunks drain at up to 3 per projection slot so the
ScalarE-bound exp backlog doesn't pile up into the endgame. Output is
written fp16 with both heads merged per i-tile, DMA split across two
queues by partition halves (host converts back to fp32).
"""

import math

import numpy as np

import concourse.mybir as mybir
import concourse.tile as tile
from concourse import bacc
from concourse.bass_utils import run_bass_kernel_spmd

# ---- problem constants (hardcoded; kernel.py must be self-contained) ----
D = 2048            # model dim (contraction for projections)
S = 2048            # sequence length per batch
NB = 2              # batches
H = 2               # heads per core
E = 128             # head dim
N_CORES = 8
IBLK = 512          # i-block (query block, matmul free dim)
JT = 128            # j-tile (key tile, partition dim)
P = 128             # partitions

FP16 = mybir.dt.float16
FP32 = mybir.dt.float32


def build_program(d=D, s=S, nb=NB, h=H, e=E, iblk=IBLK):
    """Build the per-core Bass program. Returns (nc, names dict)."""
    kd = d // P                 # contraction chunks
    st = nb * s                 # total rows of x (batches flattened)
    n_sblk = st // iblk         # projection s-blocks
    sb_per_batch = s // iblk    # i-blocks per batch
    jt_per_batch = s // JT      # j-tiles per batch
    it_per_blk = iblk // P      # i-tiles per i-block
    ew = h * e                  # projection output width per core (both heads)
    vw = h * (e + 1)            # V tile width incl. ones columns

    nc = bacc.Bacc(None, target_bir_lowering=False)
    names = {}

    with tile.TileContext(nc) as tc:
        with tc.tile_pool(name="dram", bufs=1, space="DRAM") as dram:
            # host-pre-tiled layouts (contiguous per SBUF partition line, so
            # DMA moves large packets): xTt[blk, p, t, c] = x[blk*iblk+c,
            # t*128+p]; w[p, t, e] = W[t*128+p, head cols]
            xTt = dram.tile([n_sblk, P, kd, iblk], FP16, kind="ExternalInput")
            # wall[p, t, :] = [wq cols | wk cols | wv cols] for chunk t:
            # per-(partition, chunk) contiguity 1.5KB, and a single stream
            # delivers all three weights in exact chunk-need order.
            wall = dram.tile([P, kd, 3 * ew], FP16, kind="ExternalInput")
            msk = dram.tile([P, 3 * iblk], FP16, kind="ExternalInput")
            out = dram.tile([st, ew], FP16, kind="ExternalOutput")
            names.update(xT=xTt.name, wall=wall.name,
                         msk=msk.name, out=out.name)

            with (
                tc.tile_pool(name="wpool", bufs=1) as wpool,
                tc.tile_pool(name="xpool", bufs=4) as xpool,
                tc.tile_pool(name="qkv", bufs=1) as qkv,
                tc.tile_pool(name="ppool", bufs=16) as ppool,
                tc.tile_pool(name="opool", bufs=6) as opool,
                tc.tile_pool(name="psA", bufs=2, space="PSUM") as psA,
                tc.tile_pool(name="psB", bufs=2, space="PSUM") as psB,
            ):
                # ---- startup loads, multi-engine issue, need-order ----
                # The 16 DMA engine contexts are shared by all queues and
                # each tops out at ~20-24 GB/s once packets are >=2KB, so
                # aggregate bandwidth is set by average in-flight packet
                # size, not queue count. Streams escalate part sizes:
                # small leading parts for low first-need latency, then
                # 2-8KB-contiguous parts for the bulk, spread over the
                # three DMA-capable sequencers (sync/scalar/gpsimd) in
                # consumption order.
                xtb0 = xpool.tile([P, kd, iblk], FP16, tag="xT", name="xtb0")
                xtb1 = xpool.tile([P, kd, iblk], FP16, tag="xT", name="xtb1")
                w_sb = wpool.tile([P, kd, 3 * ew], FP16, tag="wall",
                                  name="wall_sb")
                # column offsets of wq / wk / wv inside a wall chunk line
                wof = {"q": 0, "k": ew, "v": 2 * ew}
                mask_sb = wpool.tile([P, 3 * iblk], FP16, tag="mask")

                # sync: all of x block-0 in escalating parts whose lumped
                # completion times track the ~1.28us/chunk consumption
                # deadlines of the fused block-0 loop, then the mask and
                # the first half of xtb1. All queues start transferring at
                # ~7.2us (end of the engine preamble).
                for t0, t1 in ((0, 1), (1, 2), (2, 4), (4, 6), (6, 8),
                               (8, 10), (10, 12), (12, 16)):
                    nc.sync.dma_start(out=xtb0[:, t0:t1, :],
                                      in_=xTt[0, :, t0:t1, :])
                nc.sync.dma_start(out=mask_sb, in_=msk[:])
                nc.sync.dma_start(out=xtb1[:, 0:kd // 2, :],
                                  in_=xTt[1, :, 0:kd // 2, :])
                # wall chunks alternate scalar/gpsimd in chunk order: each
                # queue owes only 192KB per 2.56us -- well under a queue's
                # ~95-110 B/ns share -- so wq/wk/wv never gate a chunk.
                # Chunk 0 is split (wq|wk on scalar, wv on gpsimd) so the
                # first qk matmul waits on only 128KB.
                nc.scalar.dma_start(out=w_sb[:, 0:1, 0:2 * ew],
                                    in_=wall[:, 0:1, 0:2 * ew])
                nc.gpsimd.dma_start(out=w_sb[:, 0:1, 2 * ew:3 * ew],
                                    in_=wall[:, 0:1, 2 * ew:3 * ew])
                for t in range(1, kd):
                    eng = nc.scalar if t % 2 == 0 else nc.gpsimd
                    eng.dma_start(out=w_sb[:, t:t + 1, :],
                                  in_=wall[:, t:t + 1, :])
                nc.gpsimd.dma_start(out=xtb1[:, kd // 2:kd, :],
                                    in_=xTt[1, :, kd // 2:kd, :])

                def load_xtb(blk, parts=2):
                    t = xpool.tile([P, kd, iblk], FP16, tag="xT",
                                   name=f"xtb{blk}")
                    step = kd // parts
                    for q in range(parts):
                        t0, t1 = q * step, (q + 1) * step
                        nc.sync.dma_start(out=t[:, t0:t1, :],
                                          in_=xTt[blk, :, t0:t1, :])
                    return t

                # ---- persistent QT/KT/V in SBUF (fp16) ----
                # qT/kT: per head, [e, st] with batches side by side.
                qT = [qkv.tile([P, st], FP16, tag=f"qT{i}", name=f"qT{i}")
                      for i in range(h)]
                kT = [qkv.tile([P, st], FP16, tag=f"kT{i}", name=f"kT{i}")
                      for i in range(h)]
                # V: per (batch, j-tile): [128 j, h*(e+1)] with ones columns.
                vt = [qkv.tile([P, vw], FP16, tag=f"v{i}", name=f"v{i}")
                      for i in range(nb * jt_per_batch)]

                # ---- PE clock pre-warm ----
                # The tensor engine's DVFS ramps 0.65 -> 1.2 -> 2.4 GHz over
                # ~3us of continuous execution. Real work can't start until
                # the first x/w chunks land (~9.5us, just after the ~7.2us
                # engine preamble), so burn that wait on dependency-free
                # dummy matmuls. All accumulate into ONE psum tile in one
                # accumulation group so they stream back-to-back instead of
                # serializing on psum-buffer reuse (~213ns each).
                warm = wpool.tile([P, ew], FP16, tag="warm")
                nc.vector.memset(warm[:], 0.5)
                pw = psB.tile([P, ew], FP32, tag="psB", name="pw")
                n_warm = 12
                for i in range(n_warm):
                    nc.tensor.matmul(pw[:], warm[:, 0:P], warm[:],
                                     start=(i == 0), stop=(i == n_warm - 1))

                # ---- block 0: fully fused chunk-major qkv ----
                # 8 concurrent PSUM accumulation chains (q0,q1,k0 in psq;
                # k1 + v-it0 + v-it1 packed into psk's three banks; v-it2/
                # v-it3 in psB) consume each x chunk for 3072 PE cycles.
                # That drops the startup DMA demand to ~250 B/ns (x is
                # reused by the v matmuls), under the ~290 B/ns the queues
                # deliver: no stall, so the DVFS clock never drops to 1.2
                # GHz mid-block (a ~2us stall costs ~6us of half-clock).
                def emit_block0():
                    psq = psA.tile([P, 3 * iblk], FP32, tag="psA",
                                   name="ps0a")
                    psk = psA.tile([P, 3 * iblk], FP32, tag="psA",
                                   name="ps0b")
                    pv2 = psB.tile([P, ew], FP32, tag="psB", name="pv2")
                    pv3 = psB.tile([P, ew], FP32, tag="psB", name="pv3")
                    first, last = 0, kd - 1
                    for t in range(kd):
                        st_, sp_ = t == first, t == last
                        # q0 | q1 | k0 (one bank-chain each in psq)
                        for g, (name, c0) in enumerate(
                                (("q", 0), ("q", e), ("k", 0))):
                            nc.tensor.matmul(
                                psq[:, g * iblk:(g + 1) * iblk],
                                w_sb[:, t, wof[name] + c0:
                                     wof[name] + c0 + e],
                                xtb0[:, t, :],
                                start=st_, stop=sp_,
                            )
                        # k1 -> psk bank 0
                        nc.tensor.matmul(
                            psk[:, 0:iblk],
                            w_sb[:, t, wof["k"] + e:wof["k"] + 2 * e],
                            xtb0[:, t, :],
                            start=st_, stop=sp_,
                        )
                        # v it0 + it1 share psk bank 1 (one chain: start on
                        # the first it0 write, stop on the last it1 write,
                        # disjoint 256-col regions -- diag-quad pattern)
                        nc.tensor.matmul(
                            psk[:, iblk:iblk + ew],
                            xtb0[:, t, 0:P],
                            w_sb[:, t, wof["v"]:wof["v"] + ew],
                            start=st_, stop=False,
                        )
                        nc.tensor.matmul(
                            psk[:, iblk + ew:iblk + 2 * ew],
                            xtb0[:, t, P:2 * P],
                            w_sb[:, t, wof["v"]:wof["v"] + ew],
                            start=False, stop=sp_,
                        )
                        # v it2 / it3 in psB tiles
                        nc.tensor.matmul(
                            pv2[:], xtb0[:, t, 2 * P:3 * P],
                            w_sb[:, t, wof["v"]:wof["v"] + ew],
                            start=st_, stop=sp_,
                        )
                        nc.tensor.matmul(
                            pv3[:], xtb0[:, t, 3 * P:4 * P],
                            w_sb[:, t, wof["v"]:wof["v"] + ew],
                            start=st_, stop=sp_,
                        )
                    # evacuation copies split scalar/vector so the psum
                    # buffers free in ~2us instead of ~8us of serial
                    # vector CASTs (block-1's projections WAR on them).
                    # pv2/pv3 first on vector: block-1 starts with vprojs
                    # whose psB buffers WAR on exactly those.
                    vsrc = [psk[:, iblk:iblk + ew],
                            psk[:, iblk + ew:iblk + 2 * ew],
                            pv2[:], pv3[:]]
                    for it in (2, 3, 0, 1):
                        v_dst = vt[it]
                        for hh in range(h):
                            nc.vector.tensor_copy(
                                v_dst[:, hh * (e + 1):hh * (e + 1) + e],
                                vsrc[it][:, hh * e:(hh + 1) * e],
                            )
                            nc.vector.memset(
                                v_dst[:, hh * (e + 1) + e:
                                      hh * (e + 1) + e + 1],
                                1.0,
                            )
                    cp = mybir.ActivationFunctionType.Copy
                    nc.scalar.activation(qT[0][:, 0:iblk], psq[:, 0:iblk],
                                         cp)
                    nc.scalar.activation(qT[1][:, 0:iblk],
                                         psq[:, iblk:2 * iblk], cp)
                    nc.scalar.activation(kT[0][:, 0:iblk],
                                         psq[:, 2 * iblk:3 * iblk], cp)
                    nc.scalar.activation(kT[1][:, 0:iblk], psk[:, 0:iblk],
                                         cp)

                # ---- projections for blocks 1+, as per-chain thunks ----
                def proj_thunks(blk, pre=None):
                    s0 = blk * iblk
                    box = {}
                    if pre is not None:
                        box["x"] = pre

                    def qk(name, hh):
                        def f():
                            if "x" not in box:
                                box["x"] = load_xtb(blk)
                            xtb = box["x"]
                            dst = {"q": qT, "k": kT}[name][hh]
                            ps = psA.tile([P, iblk], FP32, tag="psA",
                                          name="ps")
                            for t in range(kd):
                                nc.tensor.matmul(
                                    ps[:],
                                    w_sb[:, t, wof[name] + hh * e:
                                         wof[name] + (hh + 1) * e],
                                    xtb[:, t, :],
                                    start=(t == 0),
                                    stop=(t == kd - 1),
                                )
                            # scalar evacuates qk psum (vector is the
                            # busier engine); the last block keeps vector
                            # so scalar's endgame exp chain stays clear
                            if blk < n_sblk - 1:
                                nc.scalar.activation(
                                    dst[:, s0:s0 + iblk], ps[:],
                                    mybir.ActivationFunctionType.Copy)
                            else:
                                nc.vector.tensor_copy(dst[:, s0:s0 + iblk],
                                                      ps[:])
                        return f

                    def vproj(it):
                        def f():
                            if "x" not in box:
                                box["x"] = load_xtb(blk)
                            xtb = box["x"]
                            ps = psB.tile([P, ew], FP32, tag="psB", name="ps")
                            for t in range(kd):
                                nc.tensor.matmul(
                                    ps[:],
                                    xtb[:, t, it * P:(it + 1) * P],
                                    w_sb[:, t, wof["v"]:wof["v"] + ew],
                                    start=(t == 0),
                                    stop=(t == kd - 1),
                                )
                            v_dst = vt[(s0 + it * P) // JT]
                            for hh in range(h):
                                nc.vector.tensor_copy(
                                    v_dst[:, hh * (e + 1):hh * (e + 1) + e],
                                    ps[:, hh * e:(hh + 1) * e],
                                )
                                nc.vector.memset(
                                    v_dst[:, hh * (e + 1) + e:
                                          hh * (e + 1) + e + 1],
                                    1.0,
                                )
                        return f

                    return ([[qk(n, hh) for n in ("q", "k")]
                             for hh in range(h)],
                            [vproj(it) for it in range(it_per_blk)])

                # ---- attention, software-pipelined ----
                # For each (batch, head, i-block) step: scores+exp for step
                # k+1 are emitted before the attn@V matmuls of step k, so the
                # PE never stalls waiting on ScalarE's exp.
                inv_sqrt_e = 1.0 / math.sqrt(e)

                def scores_thunks(b, hh, ib, p_tiles):
                    """Per-group thunks for one i-block's scores+exp+mask.

                    Full j-tiles go three-per-PSUM-tile (3 banks; one wide
                    exp covers all three). The 4 narrowed diagonal j-tiles
                    pack into ONE 3-bank tile: bank0 = d0[512], bank1 =
                    d1[384] + d3[128] (one accumulation group, disjoint
                    writes), bank2 = d2[256]; a single exp + one host-built
                    mask handle the whole diagonal. Each thunk appends
                    per-jt (p_tile, eff) entries to p_tiles; the PV lhsT
                    slice for i-tile t is p_tile[:, t*128+eff :][:128].
                    """
                    i0 = b * s + ib * iblk
                    n_full = it_per_blk * ib

                    def score_mm(sp, base, jt, c0, start=True, stop=True):
                        nc.tensor.matmul(
                            sp[:, base:base + (iblk - c0)],
                            kT[hh][:, b * s + jt * JT:b * s + (jt + 1) * JT],
                            qT[hh][:, i0 + c0:i0 + iblk],
                            start=start,
                            stop=stop,
                        )

                    def full_group(g0):
                        def f():
                            gn = min(3, n_full - g0)
                            sp = psA.tile([P, 3 * iblk], FP32, tag="psA",
                                          name="sp")
                            pt = ppool.tile([P, 3 * iblk], FP16, tag="p",
                                            name="pt")
                            for k in range(gn):
                                score_mm(sp, k * iblk, g0 + k, 0)
                                p_tiles.append((pt, k * iblk))
                            nc.scalar.activation(
                                pt[:, 0:gn * iblk], sp[:, 0:gn * iblk],
                                mybir.ActivationFunctionType.Exp,
                                scale=inv_sqrt_e,
                            )
                        return f

                    def diag_quad():
                        sp = psA.tile([P, 3 * iblk], FP32, tag="psA",
                                      name="sp")
                        pt = ppool.tile([P, 3 * iblk], FP16, tag="p",
                                        name="pt")
                        q0 = n_full
                        score_mm(sp, 0, q0 + 0, 0)                # d0 [0:512]
                        score_mm(sp, iblk, q0 + 1, P, stop=False)
                        score_mm(sp, iblk + 384, q0 + 3, 3 * P,
                                 start=False)                     # d3
                        score_mm(sp, 2 * iblk, q0 + 2, 2 * P)     # d2
                        p_tiles.append((pt, 0))                   # d0: eff 0
                        p_tiles.append((pt, iblk - P))            # d1: eff 384
                        p_tiles.append((pt, 2 * iblk - 2 * P))    # d2: eff 768
                        p_tiles.append((pt, iblk + 384 - 3 * P))  # d3: eff 512
                        tw = 2 * iblk + 256
                        nc.scalar.activation(
                            pt[:, 0:tw], sp[:, 0:tw],
                            mybir.ActivationFunctionType.Exp,
                            scale=inv_sqrt_e,
                        )
                        nc.vector.tensor_mul(
                            pt[:, 0:tw], pt[:, 0:tw], mask_sb[:, 0:tw]
                        )

                    return ([full_group(g0) for g0 in range(0, n_full, 3)]
                            + [diag_quad])

                # merged per-i-tile output tiles: both heads' normalized
                # rows land in one [128, 256] fp16 tile, DMA'd once (split
                # across two queues by partition halves).
                obt_map = {}

                def pv_thunks(b, hh, ib, p_tiles):
                    i0 = b * s + ib * iblk
                    jbase = b * jt_per_batch

                    def one(it):
                        def f():
                            op = psB.tile([P, e + 1], FP32, tag="psB",
                                          name="op")
                            last = it_per_blk * ib + it
                            for jt in range(last + 1):
                                pt, eff = p_tiles[jt]
                                lo = it * P + eff
                                nc.tensor.matmul(
                                    op[:],
                                    pt[:, lo:lo + P],
                                    vt[jbase + jt][:, hh * (e + 1):
                                                   (hh + 1) * (e + 1)],
                                    start=(jt == 0),
                                    stop=(jt == last),
                                )
                            rec = opool.tile([P, 1], FP32, tag="rec",
                                             name="rec")
                            key = (b, ib, it)
                            ent = obt_map.get(key)
                            if ent is None:
                                obt = opool.tile([P, ew], FP16, tag="obt",
                                                 bufs=12, name="obt")
                                ent = obt_map[key] = [obt, 0]
                            obt = ent[0]
                            nc.vector.reciprocal(rec[:], op[:, e:e + 1])
                            nc.vector.tensor_scalar_mul(
                                obt[:, hh * e:(hh + 1) * e], op[:, 0:e],
                                rec[:])
                            ent[1] += 1
                            if ent[1] == h:
                                r0 = i0 + it * P
                                # the last i-block's tiles go on sync +
                                # scalar in halves (queues empty by then,
                                # HW drains ~10ns, parallel transfer);
                                # everything earlier stays on gpsimd,
                                # whose software-DGE drain polls ~3us past
                                # the final transfer -- fine mid-run,
                                # deadly in the teardown.
                                if (b, ib) == (nb - 1, sb_per_batch - 1):
                                    eng = nc.sync if it % 2 == 0 else nc.scalar
                                    eng.dma_start(
                                        out=out[r0:r0 + P, :],
                                        in_=obt[:],
                                    )
                                else:
                                    nc.gpsimd.dma_start(
                                        out=out[r0:r0 + P, :],
                                        in_=obt[:],
                                    )
                                del obt_map[key]
                        return f

                    return [one(it) for it in range(it_per_blk)]

                # ---- interleaved emission ----
                # Attention step (b, hh, ib) becomes ready once projection
                # s-block b*sb_per_batch+ib is emitted. Its score-group
                # thunks are queued immediately, its attn@V thunks one step
                # later (so scores of the next step always precede attn@V of
                # the previous -> no exp-latency stall). Between projection
                # thunks, 1-3 attention thunks are emitted depending on
                # backlog, so the ScalarE-bound endgame stays small.
                from collections import deque

                attn_q = deque()
                pending_pv = None   # (thunks, step_block) of the last step
                step_list = sorted(
                    [(b, hh, ib) for b in range(nb) for hh in range(h)
                     for ib in range(sb_per_batch)],
                    key=lambda st: (st[0] * sb_per_batch + st[2], st[1]),
                )
                si = 0

                def queue_ready(blk_done, hh_done=None):
                    # admit steps whose q/k projections are emitted; with
                    # hh_done set, only heads <= hh_done of block blk_done
                    nonlocal si, pending_pv
                    def admitted(st):
                        sb = st[0] * sb_per_batch + st[2]
                        if sb < blk_done:
                            return True
                        return sb == blk_done and (hh_done is None
                                                   or st[1] <= hh_done)
                    while (si < len(step_list)
                           and admitted(step_list[si])):
                        st = step_list[si]
                        si += 1
                        st_blk = st[0] * sb_per_batch + st[2]
                        shared = []
                        sc = [("sc", t, None)
                              for t in scores_thunks(*st, shared)]
                        pv = ([("pv", t, pending_pv[1])
                               for t in pending_pv[0]]
                              if pending_pv is not None else [])
                        # zip score-groups with the previous step's attn@V
                        # thunks: each attn@V group is ~1.5us of PE work that
                        # covers the exp latency of the preceding scores.
                        # Scores advance at ~2x rate so the LAST (diag)
                        # group lands before the final attn@V thunks: its
                        # exp completes under them instead of stalling the
                        # next step's first attn@V.
                        merged = []
                        if sc and pv:
                            merged.extend([sc[0], pv[0]])
                            rest_sc, i2 = sc[1:], 0
                            for p_ in pv[1:]:
                                merged.extend(rest_sc[i2:i2 + 2])
                                i2 += 2
                                merged.append(p_)
                            merged.extend(rest_sc[i2:])
                        else:
                            merged = sc + pv
                        attn_q.extend(merged)
                        pending_pv = (pv_thunks(*st, shared), st_blk)

                def pop_some(vphase_blk=None):
                    npop = (5 if len(attn_q) > 24
                            else 4 if len(attn_q) > 16
                            else 3 if len(attn_q) > 8
                            else 2 if len(attn_q) > 2 else 1)
                    sc_run = 0
                    for _ in range(npop):
                        if not attn_q:
                            break
                        kind, fn, pv_blk = attn_q[0]
                        # cap back-to-back score groups per slot: the
                        # 2-deep psA pool stalls the PE on a 3rd
                        if kind == "sc":
                            sc_run += 1
                            if sc_run > 2:
                                break
                        # a pv whose step belongs to the block whose
                        # V-projections are being emitted right now would
                        # be ordered before the V tiles it reads
                        elif vphase_blk is not None and pv_blk == vphase_blk:
                            break
                        attn_q.popleft()
                        fn()

                # steps become ready right after their block's qk thunks:
                # their scores+exp overlap the block's V-projections, so
                # the ScalarE exp backlog starts draining ~7us earlier per
                # block and the endgame exp chain is mostly prepaid.
                emit_block0()
                queue_ready(0)
                pre_x = {1: xtb1}
                for blk in range(1, n_sblk):
                    qk_thunks, v_thunks = proj_thunks(blk,
                                                      pre=pre_x.get(blk))
                    # v it0/it1 first: their psB buffers clear fastest
                    # after the previous block, so the PE restarts without
                    # waiting on the bigger psA evacuations. Each head's
                    # attention step is admitted right after its own q/k
                    # thunks, so its scores+exp overlap the rest of the
                    # block (prepaying the ScalarE exp chain).
                    for th in v_thunks[:2]:
                        th()
                        pop_some(vphase_blk=blk)
                    for hh, pair in enumerate(qk_thunks):
                        for th in pair:
                            th()
                            pop_some()
                        queue_ready(blk, hh_done=hh)
                    for th in v_thunks[2:]:
                        th()
                        pop_some(vphase_blk=blk)
                while attn_q:
                    attn_q.popleft()[1]()
                if pending_pv is not None:
                    for th in pending_pv[0]:
                        th()

    nc.compile()
    return nc, names


def host_tile_x(x_flat, iblk, p=P):
    """[st, d] -> [n_sblk, p, kd, iblk] with layout x[blk*iblk+c, t*p+pp]."""
    st, d = x_flat.shape
    return np.ascontiguousarray(
        x_flat.reshape(st // iblk, iblk, d // p, p).transpose(0, 3, 2, 1)
        .astype(np.float16)
    )


def host_tile_w(w_cols, p=P):
    """[d, ew] -> [p, kd, ew] with layout W[t*p+pp, e]."""
    d, ew = w_cols.shape
    return np.ascontiguousarray(
        w_cols.reshape(d // p, p, ew).transpose(1, 0, 2).astype(np.float16)
    )


def host_mask(iblk, p=P):
    """Causal mask [p, 3*iblk] for the packed diagonal quad layout:
    cols [0:512]=d0, [512:896]=d1(384), [896:1024]=d3(128), [1024:1280]=d2
    (256). Every narrowed diagonal tile reduces to the base pattern
    diag[pp, c] = (pp <= c)."""
    diag = (np.arange(p)[:, None] <= np.arange(iblk)[None, :])
    m = np.zeros((p, 3 * iblk), dtype=np.float16)
    m[:, 0:iblk] = diag
    m[:, iblk:iblk + 384] = diag[:, 0:384]
    m[:, iblk + 384:iblk + 512] = diag[:, 0:128]
    m[:, 2 * iblk:2 * iblk + 256] = diag[:, 0:256]
    return m


def _host_prep(x, Wq, Wk, Wv):
    """Shard + cast inputs on host. Returns list of 8 in_maps."""
    st = x.shape[0] * x.shape[1]
    xTt = host_tile_x(x.reshape(st, D), IBLK)
    msk = host_mask(IBLK)
    in_maps = []
    for c in range(N_CORES):
        cols = slice(2 * c * E, 2 * (c + 1) * E)
        wall = np.ascontiguousarray(np.concatenate(
            [host_tile_w(Wq[:, cols]), host_tile_w(Wk[:, cols]),
             host_tile_w(Wv[:, cols])], axis=2))
        in_maps.append({
            "xT": xTt,
            "wall": wall,
            "msk": msk,
        })
    return in_maps


_CACHE = {}


def _get_program():
    if "nc" not in _CACHE:
        nc, names = build_program()
        _CACHE["nc"] = nc
        _CACHE["names"] = names
    return _CACHE["nc"], _CACHE["names"]


def kernel(x, Wq, Wk, Wv, _trace=False, _tmpdir=None):
    nc, names = _get_program()
    raw_maps = _host_prep(np.asarray(x), np.asarray(Wq), np.asarray(Wk),
                          np.asarray(Wv))
    in_maps = [{names[k]: v for k, v in m.items()} for m in raw_maps]
    res = run_bass_kernel_spmd(
        nc, in_maps, core_ids=list(range(N_CORES)),
        trace=_trace, tmpdir=_tmpdir,
    )
    b, s, d = x.shape
    out = np.empty((b, s, d), dtype=np.float32)
    for c in range(N_CORES):
        core_out = res.results[c][names["out"]]  # [4096, 256] fp16
        out[:, :, 2 * c * E:2 * (c + 1) * E] = (
            core_out.astype(np.float32).reshape(b, s, 2 * E))
    if _trace:
        _CACHE["last_results"] = res
    return out

